# revision 30
# baseline (speedup 1.0000x reference)
"""Trainium2 Bass kernel for CausalPriorityAttention.

Data-parallel over the batch dim: core b computes batch b (B=8, 8 cores).

v4: fp16 dataflow end-to-end, DMA dispatch spread across queues, x^T and
W^T host-packed into one dram tensor, and a software-pipelined emission
schedule tuned for the in-order engine queues:
  - pair-0's score matmuls are emitted right after pair-0's QKV groups so
    ACT flows from tanh/E straight into score exps with no gap;
  - pair p+1's scores are emitted before pair p's PV (lookahead);
  - pair-0's probs multiplies are deferred past phase-1's DVE copy-outs;
  - the NEXT rep's input DMAs, tanh/E pass, pair-0 QKV groups and pair-0
    scores+exps are emitted inside the current rep's tail (they only
    touch buffers that free early: e_sb is double-buffered per rep), so
    in the R-rep steady state ACT never waits for a rep boundary.

Per-core dataflow (512-wide matmuls, fp16 operands, f32 psum):
  phase 1: qkT = W_qk @ x^T  (Q^T,K^T in [feat, seq] layout)
           V   = x @ W_v^T   (natural [seq, feat] layout, +ones col)
           E   = exp(5a*tanh(x/2) + 5a) = exp(10a*sigmoid(x)); Tanh lives
           in the same ACT table set as Exp => zero table reloads
  phase 2 (per head pair, row-group-paired K=64 score matmuls):
           sT[k,q] = K @ Q'^T            (transposed scores -> PSUM)
           probs   = exp(10a*sT - 5) * E (ACT exp + DVE fp16 mult)
           pv[65,q] = [V_h | 1]^T @ probs  (out^T rows + rowsums)
           attnT = pv[0:64] * recip(pv[64])  (partition_broadcast)
  phase 3: y = attnT^T @ Wo^T + bo, emitted per q-chunk as soon as the
           last head pair's normalization for it lands
Q' is prescaled by 1/(8*10a) so exp's scale=10a restores QK/8; the
reference bias's constant -5a term drops out of softmax, and the -5 shift
(which also cancels in normalization) keeps exp products in fp16 range.
The transposed-score layout means graph_bias is consumed untransposed and
probs come out pre-transposed for the PV matmul: zero on-chip transposes.
"""

import sys

for _p in ("/opt/trn_rl_repo",):
    if _p not in sys.path:
        sys.path.append(_p)

import numpy as np

import concourse.bacc as bacc
import concourse.bass as bass
import concourse.mybir as mybir
import concourse.tile as tile
from concourse.bass_utils import run_bass_kernel_spmd

B, N, D = 8, 1024, 512
H, HD = 8, 64
P = 128
NT = N // P          # 8 seq tiles
KT = D // P          # 4 contraction tiles over D
FT_QK = 2 * D // P   # 8 feature tiles over [Q;K]
QC = N // 512        # 2 q-chunks of 512
XW = N + 3 * D       # 2560 cols of host-packed [x^T | W^T]
F32 = mybir.dt.float32
F16 = mybir.dt.float16

_CACHE = {}


def build_nc(ten_a: float, reps: int = 1):
    nc = bacc.Bacc("TRN2")
    xw = nc.dram_tensor("xw", [D, XW], F16, kind="ExternalInput")
    gb = nc.dram_tensor("gb", [N, N], F16, kind="ExternalInput")
    woT = nc.dram_tensor("woT", [D, D], F16, kind="ExternalInput")
    qkb = nc.dram_tensor("qkb", [P, FT_QK], F32, kind="ExternalInput")
    vb = nc.dram_tensor("vb", [D], F16, kind="ExternalInput")
    bo = nc.dram_tensor("bo", [1, D], F16, kind="ExternalInput")
    ones1 = nc.dram_tensor("ones1", [1, P], F16, kind="ExternalInput")
    y = nc.dram_tensor("y", [N, D], F32, kind="ExternalOutput")

    sQ = 1.0 / (8.0 * ten_a)

    with tile.TileContext(nc) as tc:
        with tc.tile_pool(name="const", bufs=1) as const_pool, \
             tc.tile_pool(name="persist", bufs=1) as persist, \
             tc.tile_pool(name="ph1", bufs=1) as ph1, \
             tc.tile_pool(name="ph2", bufs=1) as ph2, \
             tc.tile_pool(name="small", bufs=2) as small:
            qkb_sb = const_pool.tile([P, FT_QK], F32)
            nc.sync.dma_start(out=qkb_sb, in_=qkb[:, :])
            vb_sb = const_pool.tile([P, D], F16)
            nc.sync.dma_start(
                out=vb_sb,
                in_=bass.AP(tensor=vb.ap().tensor, offset=0, ap=[[0, P], [1, D]]),
            )
            bo_sb = const_pool.tile([1, D], F16)
            nc.sync.dma_start(out=bo_sb, in_=bo[:, :])
            ones1_sb = const_pool.tile([1, P], F16)
            nc.sync.dma_start(out=ones1_sb, in_=ones1[:, :])
            neg5 = const_pool.tile([P, 1], F32)
            nc.vector.memset(neg5, -5.0)
            p5a = const_pool.tile([P, 1], F32)
            nc.vector.memset(p5a, ten_a / 2.0)

            qkT_sb = persist.tile([P, FT_QK, N], F16)
            v_sb = persist.tile([P, NT, H, HD + 1], F16)
            nc.gpsimd.memset(v_sb[:, :, :, HD : HD + 1], 1.0)
            # double-buffered per rep so the next rep's E pass can run in
            # this rep's ACT tail while this rep's multiplies still read E
            e_tiles = [persist.tile([P, NT, N], F16, name="e0"),
                       persist.tile([P, NT, N], F16, name="e1")]
            attnT_sb = persist.tile([P, KT, N], F16)
            woT_sb = persist.tile([P, KT, D], F16)
            nc.gpsimd.dma_start(
                out=woT_sb, in_=woT[:, :].rearrange("(t p) n -> p t n", p=P)
            )

            # ---------- emission helpers ----------
            def emit_in_dmas(eb):
                """xw + graph_bias DMAs and the tanh/E ACT pass for one rep.
                Returns xw_sb."""
                xw_sb = ph1.tile([P, KT, XW], F16, tag="xw", bufs=1,
                                 name="xw_sb")
                for k in range(KT):
                    nc.sync.dma_start(
                        out=xw_sb[:, k, :], in_=xw[k * P : (k + 1) * P, :]
                    )
                gts = []
                for k2 in range(NT // 2):
                    gt = ph1.tile([P, 2, N], F16, tag="gbt2", bufs=2,
                                  name="gbt2")
                    for j in range(2):
                        kt = 2 * k2 + j
                        nc.gpsimd.dma_start(
                            out=gt[:, j, :], in_=gb[kt * P : (kt + 1) * P, :]
                        )
                    gts.append(gt)
                sgs = []
                for k2 in range(NT // 2):
                    sg = ph1.tile([P, 2, N], F16, tag="sig2", bufs=2,
                                  name="sig2")
                    nc.scalar.activation(
                        out=sg, in_=gts[k2],
                        func=mybir.ActivationFunctionType.Tanh,
                        scale=0.5,
                    )
                    sgs.append(sg)
                for k2 in range(NT // 2):
                    nc.scalar.activation(
                        out=eb[:, 2 * k2 : 2 * k2 + 2, :],
                        in_=sgs[k2],
                        func=mybir.ActivationFunctionType.Exp,
                        scale=ten_a / 2.0,
                        bias=p5a,
                    )
                return xw_sb

            def emit_blocks(groups, xw_sb, alloc):
                """QKV projection groups, k outermost within blocks of 2 psum
                groups so k=0..2 matmuls run as each xw DMA chunk lands.
                Blocks draw [P,512] tiles from the PV accumulator ring (its
                frees are DVE-paced copy-outs, never ACT-paced), so phase 1
                needs no psum pool of its own."""
                for b0 in range(0, len(groups), 2):
                    blk = groups[b0 : b0 + 2]
                    tiles = [alloc() for _ in blk]
                    for k in range(KT):
                        for g, t in zip(blk, tiles):
                            if g[0] == "qk":
                                _, ft, qc = g
                                nc.tensor.matmul(
                                    t,
                                    lhsT=xw_sb[
                                        :, k, N + ft * P : N + (ft + 1) * P
                                    ],
                                    rhs=xw_sb[:, k, qc * 512 : (qc + 1) * 512],
                                    start=(k == 0),
                                    stop=(k == KT - 1),
                                )
                            else:
                                _, st, _ = g
                                nc.tensor.matmul(
                                    t,
                                    lhsT=xw_sb[:, k, st * P : (st + 1) * P],
                                    rhs=xw_sb[:, k, N + 2 * D : N + 3 * D],
                                    start=(k == 0),
                                    stop=(k == KT - 1),
                                )
                    for g, t in zip(blk, tiles):
                        if g[0] == "qk":
                            _, ft, qc = g
                            # copy out with per-partition bias + Q prescale
                            nc.vector.tensor_scalar(
                                out=qkT_sb[:, ft, qc * 512 : (qc + 1) * 512],
                                in0=t,
                                scalar1=qkb_sb[:, ft : ft + 1],
                                scalar2=(sQ if ft < FT_QK // 2 else 1.0),
                                op0=mybir.AluOpType.add,
                                op1=mybir.AluOpType.mult,
                            )
                        else:
                            _, st, _ = g
                            nc.vector.tensor_tensor(
                                out=v_sb[:, st, :, 0:HD],
                                in0=t.rearrange("p (h d) -> p h d", h=H),
                                in1=vb_sb.rearrange("p (h d) -> p h d", h=H),
                                op=mybir.AluOpType.add,
                            )

            def g_pair(hp):
                return [("qk", ft, qc) for ft in (hp, 4 + hp)
                        for qc in range(QC)]

            G_V = [("v", st, 0) for st in range(NT)]

            def emit_scores_exps(hp, ps_s):
                """Score matmuls + ACT exps for head pair hp. DVE multiplies
                are emitted separately (emit_mults) for DVE queue order."""
                es_tiles = []
                for kt in range(NT):
                    sT2 = [
                        ps_s.tile([P, N], F32, tag="sT", name="sTa"),
                        ps_s.tile([P, N], F32, tag="sT", name="sTb"),
                    ]
                    for qc in range(QC):
                        for sub in range(2):
                            qp = 64 * sub
                            nc.tensor.matmul(
                                sT2[sub][:, qc * 512 : (qc + 1) * 512],
                                lhsT=qkT_sb[
                                    qp : qp + HD,
                                    FT_QK // 2 + hp,
                                    kt * P : (kt + 1) * P,
                                ],
                                rhs=qkT_sb[
                                    qp : qp + HD, hp, qc * 512 : (qc + 1) * 512
                                ],
                                start=True,
                                stop=True,
                            )
                    for sub in range(2):
                        # -5 keeps exp(s)*exp(bias) products in fp16 range;
                        # the shift cancels in normalization
                        es = ph2.tile([P, N], F16, tag="es", bufs=8, name="es")
                        nc.scalar.activation(
                            out=es,
                            in_=sT2[sub],
                            func=mybir.ActivationFunctionType.Exp,
                            scale=ten_a,
                            bias=neg5,
                        )
                        es_tiles.append(es)
                return es_tiles

            def new_expT():
                return [
                    ph2.tile([P, NT, N], F16, tag="exp0", bufs=2, name="expT0"),
                    ph2.tile([P, NT, N], F16, tag="exp1", bufs=2, name="expT1"),
                ]

            def emit_mults(es_tiles, expT, eb, lo=0, hi=2 * NT):
                for i in range(lo, hi):
                    kt, sub = divmod(i, 2)
                    nc.vector.tensor_tensor(
                        out=expT[sub][:, kt, :],
                        in0=es_tiles[i],
                        in1=eb[:, kt, :],
                        op=mybir.AluOpType.mult,
                    )

            def emit_pv_norm(hp, expT, qc, ps_acc):
                for sub in range(2):
                    h = 2 * hp + sub
                    qp = 64 * sub
                    acc = ps_acc.tile([P, 512], F32, tag="acc", name="acc")
                    pv = acc[0 : HD + 1, :]
                    for kt in range(NT):
                        nc.tensor.matmul(
                            pv,
                            lhsT=v_sb[:, kt, h, :],
                            rhs=expT[sub][:, kt, qc * 512 : (qc + 1) * 512],
                            start=(kt == 0),
                            stop=(kt == NT - 1),
                        )
                    attn_out = attnT_sb[
                        qp : qp + HD, hp, qc * 512 : (qc + 1) * 512
                    ]
                    recip = small.tile([1, 512], F32, tag="recip", name="recip")
                    nc.vector.reciprocal(recip, pv[HD : HD + 1, :])
                    bc = small.tile([HD, 512], F32, tag="bc", name="bc")
                    nc.gpsimd.partition_broadcast(bc, recip)
                    nc.vector.tensor_tensor(
                        out=attn_out,
                        in0=pv[0:HD, :],
                        in1=bc,
                        op=mybir.AluOpType.mult,
                    )

            def emit_ph3(qc, ps_acc):
                for st in range(qc * NT // 2, (qc + 1) * NT // 2):
                    yp = ps_acc.tile([P, D], F32, tag="acc", name="yp")
                    for ft in range(KT):
                        nc.tensor.matmul(
                            yp,
                            lhsT=attnT_sb[:, ft, st * P : (st + 1) * P],
                            rhs=woT_sb[:, ft, :],
                            start=(ft == 0),
                            stop=False,
                        )
                    # rank-1 ones-row matmul adds bo into the psum
                    nc.tensor.matmul(
                        yp, lhsT=ones1_sb, rhs=bo_sb, start=False, stop=True
                    )
                    ysb = ph2.tile([P, D], F32, tag="ysb", bufs=2, name="ysb")
                    nc.vector.tensor_scalar(
                        out=ysb, in0=yp, scalar1=1.0, scalar2=None,
                        op0=mybir.AluOpType.mult,
                    )
                    nc.gpsimd.dma_start(
                        out=y[st * P : (st + 1) * P, :], in_=ysb
                    )

            # ---------- software-pipelined rep loop ----------
            # Emission order IS the per-engine program order; this schedule
            # keeps every in-order queue fed:
            #  - pair-1 scores right after pair-1's blocks (ACT flows from
            #    the rep-boundary exps p0 straight into exps p1);
            #  - pair-0 multiplies chunk-interleaved with phase-1 copy-outs
            #    on DVE (mults free es slots for ACT; copies chase PE);
            #  - remaining blocks DVE/acc-paced, never ACT-paced;
            #  - the next rep's prologue (DMAs, tanh/E, pair-0 blocks +
            #    scores + exps) emitted between PV p1 and PV p2 so ACT
            #    rolls over the rep boundary without a gap.
            # Schedule (measured best on the cost model): phase 1 as one
            # lump with pair-0's blocks+scores leading it, lookahead hp
            # loop (scores p+1 before PV p), next rep's prologue at hp==2.
            # Variants that hoisted pair-1's scores earlier, interleaved
            # blocks per pair, or window-balanced the PE filler all
            # measured worse on the 4-rep steady-state slope.
            G_REST = g_pair(1) + g_pair(2) + g_pair(3) + G_V
            es0 = expT0 = xw_cur = None
            for r in range(reps):
                eb = e_tiles[r % 2]
                if r == 0:
                    ps_s_cm = tc.tile_pool(name="ps_s", bufs=3, space="PSUM")
                    ps_s = ps_s_cm.__enter__()
                    ps1_cm = tc.tile_pool(name="ps1", bufs=2, space="PSUM")
                    ps1 = ps1_cm.__enter__()
                else:
                    ps1_cm = tc.tile_pool(name="ps1", bufs=4, space="PSUM")
                    ps1 = ps1_cm.__enter__()

                def a1():
                    return ps1.tile([P, 512], F32, tag="ps1", name="ps")

                if r == 0:
                    xw_cur = emit_in_dmas(eb)
                    emit_blocks(g_pair(0), xw_cur, a1)
                    expT0 = new_expT()
                    es0 = emit_scores_exps(0, ps_s)
                emit_blocks(G_REST, xw_cur, a1)
                ps1_cm.__exit__(None, None, None)
                if r > 0:
                    ps_s_cm = tc.tile_pool(name="ps_s", bufs=3, space="PSUM")
                    ps_s = ps_s_cm.__enter__()

                with tc.tile_pool(name="ps_acc", bufs=2,
                                  space="PSUM") as ps_acc:
                    def a2():
                        return ps_acc.tile([P, 512], F32, tag="acc",
                                           name="ps")
                    emit_mults(es0, expT0, eb)
                    expT = {0: expT0}
                    for hp in range(H // 2):
                        nxt = hp + 1
                        es_n = None
                        if nxt < H // 2:
                            es_n = emit_scores_exps(nxt, ps_s)
                            expT[nxt] = new_expT()
                        if hp == H // 2 - 2 and r + 1 < reps:
                            # next rep's prologue, hidden in this rep's tail
                            eb2 = e_tiles[(r + 1) % 2]
                            xw_cur = emit_in_dmas(eb2)
                            emit_blocks(g_pair(0), xw_cur, a2)
                            expT0 = new_expT()
                            es0 = emit_scores_exps(0, ps_s)
                        if hp < H // 2 - 1:
                            for qc in range(QC):
                                emit_pv_norm(hp, expT[hp], qc, ps_acc)
                            if es_n is not None:
                                emit_mults(es_n, expT[nxt], eb)
                        else:
                            for qc in range(QC):
                                emit_pv_norm(hp, expT[hp], qc, ps_acc)
                                emit_ph3(qc, ps_acc)
                ps_s_cm.__exit__(None, None, None)
    nc.finalize()
    return nc


def kernel(x, graph_bias, in_proj_w, in_proj_b, out_proj_w, out_proj_b,
           bias_strength):
    x = np.asarray(x, dtype=np.float32)
    graph_bias = np.asarray(graph_bias, dtype=np.float32)
    in_proj_w = np.asarray(in_proj_w, dtype=np.float32)
    in_proj_b = np.asarray(in_proj_b, dtype=np.float32)
    out_proj_w = np.asarray(out_proj_w, dtype=np.float32)
    out_proj_b = np.asarray(out_proj_b, dtype=np.float32)
    alpha = 1.0 / (1.0 + np.exp(-float(np.asarray(bias_strength))))
    ten_a = 10.0 * alpha

    key = round(ten_a, 9)
    if key not in _CACHE:
        _CACHE[key] = build_nc(ten_a)
    nc = _CACHE[key]

    wT = in_proj_w.T                                # [512, 1536]
    woT = np.ascontiguousarray(out_proj_w.T).astype(np.float16)
    qkb = np.ascontiguousarray(
        in_proj_b[: 2 * D].reshape(FT_QK, P).T      # [128, 8]
    )
    vb = in_proj_b[2 * D :].astype(np.float16)
    bo = out_proj_b.astype(np.float16)
    gb16 = graph_bias.astype(np.float16)

    in_maps = []
    for b in range(B):
        xwb = np.concatenate([x[b].T, wT], axis=1).astype(np.float16)
        in_maps.append({
            "xw": np.ascontiguousarray(xwb),
            "gb": np.ascontiguousarray(gb16[b]),
            "woT": woT,
            "qkb": qkb,
            "vb": vb,
            "bo": bo.reshape(1, D),
            "ones1": np.ones((1, P), dtype=np.float16),
        })

    global _saved_in_maps
    _saved_in_maps = in_maps
    res = run_bass_kernel_spmd(nc, in_maps, core_ids=list(range(B)))
    out = np.stack([res.results[b]["y"] for b in range(B)], axis=0)
    return out.astype(np.float32)
